# revision 1
# baseline (speedup 1.0000x reference)
"""MultiHeadAttention (cross-attention, B=32 N=512 L=1024 D=512 H=8) on 8 TRN2 cores.

Strategy: data parallelism (4 batches/core) + host-side sparsity compaction.

Host prep (inside kernel(), plain numpy):
  - per batch, gather the unmasked K/V positions (~50% of L=1024), pad to
    L_C=640 (5*128); padded slots get zero K/V rows and a -87 exp bias so they
    vanish from the softmax exactly like reference's -inf masking
  - rpb rows gathered the same way; x_q / x_kv / rpb pre-TRANSPOSED on host so
    the device needs no PE transposes at all
Device per-core dataflow (all matmuls float32r, 1 cycle/row on PE):
  Q^T/K^T (+rpb^T via DVE add) head-major; V natural with interleaved ones col
  scores S^T[l,n] per head-pair packed via tile_position (K=64 row groups),
  both heads' scores in one [128,1024] PSUM tile -> single exp per (pair,chunk)
  exp on ACT with per-partition bias (pad masking; no max subtraction needed)
  stage2 O^T[c,n] = [V|1]^T @ P^T accumulated over l chunks (heads interleaved
  so P^T tiles release early); row 64 = softmax denominator
  normalize via reciprocal + gpsimd partition_broadcast, o_proj to natural
  layout, + bias, DMA out.
Emission is software-pipelined: prep (DMAs + QKV projections) of batch b+1 is
interleaved into the ACT-bound attention phase of batch b.
"""
import sys

sys.path.insert(0, "/opt/trn_rl_repo")
import numpy as np

B, N, L, D, H, C = 32, 512, 1024, 512, 8, 64
NCORES = 8
BLOC = B // NCORES  # 4 batches per core
SCALE = C ** -0.5
MASK_NEG = -87.0
P = 128
NDC = D // P   # 4 d/e chunks
NNC = N // P   # 4 n chunks
LC_SPARSE = 640

_CACHE = {}


def _nspans(l_c):
    # split l_c into moving-operand spans <=512, each >=256 (f32r full rate)
    if l_c == 640:
        return [(0, 384), (384, 640)]
    return [(s, min(s + 512, l_c)) for s in range(0, l_c, 512)]


def _build_nc(l_chunks):
    import concourse.bacc as bacc
    import concourse.tile as tile
    from concourse import mybir

    f32 = mybir.dt.float32
    f32r = mybir.dt.float32r
    EXP = mybir.ActivationFunctionType.Exp
    L_C = l_chunks * P

    nc = bacc.Bacc()
    xqT_d = nc.declare_dram_parameter("xqT", [BLOC, D, N], f32r, isOutput=False)
    xkT_d = nc.declare_dram_parameter("xkT", [BLOC, D, L_C], f32r, isOutput=False)
    rpbT_d = nc.declare_dram_parameter("rpbT", [BLOC, D, L_C], mybir.dt.bfloat16, isOutput=False)
    mb_d = nc.declare_dram_parameter("mbias", [BLOC, L_C], f32, isOutput=False)
    Wq = nc.declare_dram_parameter("Wq", [D, D], f32r, isOutput=False)
    Wk = nc.declare_dram_parameter("Wk", [D, D], f32r, isOutput=False)
    Wv = nc.declare_dram_parameter("Wv", [D, D], f32r, isOutput=False)
    Wo = nc.declare_dram_parameter("Wo", [D, D], f32r, isOutput=False)
    bo = nc.declare_dram_parameter("bo", [1, D], f32, isOutput=False)
    out = nc.declare_dram_parameter("out", [BLOC, N, D], f32, isOutput=True)

    sparse = l_chunks <= 5
    with tile.TileContext(nc) as tc:
        with (
            tc.tile_pool(name="consts", bufs=1) as consts,
            tc.tile_pool(name="xin", bufs=2 if sparse else 1) as xin_pool,
            tc.tile_pool(name="qkt", bufs=2 if sparse else 1) as qkt_pool,
            tc.tile_pool(name="vp", bufs=2 if sparse else 1) as vp_pool,
            tc.tile_pool(name="pt", bufs=7 if sparse else 4) as pt_pool,
            tc.tile_pool(name="ot", bufs=2) as ot_pool,
            tc.tile_pool(name="outst", bufs=3) as outst_pool,
            tc.tile_pool(name="small", bufs=2) as small,
            tc.tile_pool(name="ps_sc", bufs=2, space="PSUM") as ps_sc,
            tc.tile_pool(name="ps_mm", bufs=4, space="PSUM") as ps_mm,
        ):
            state = {}

            # ---- one-time setup ----
            warm = consts.tile([P, 1], f32, tag="warm")
            nc.vector.memset(warm, 0.0)
            nc.scalar.activation(out=warm, in_=warm, func=EXP, scale=1.0)

            ones8 = consts.tile([P, H], f32, tag="ones8")
            nc.vector.memset(ones8, 1.0)

            wsb = {}

            def load_w(wi, W):
                for k in range(NDC):
                    wt = consts.tile([P, D], f32r, tag=f"w{wi}_{k}", name=f"w{wi}_{k}")
                    nc.sync.dma_start(out=wt, in_=W[k * P:(k + 1) * P, :])
                    wsb[(wi, k)] = wt

            # Wq and batch-0 xqT interleaved by chunk so the first Q-proj
            # matmul group can start after ~0.5 MB of DMA
            xqT0 = []
            for k in range(NDC):
                wt = consts.tile([P, D], f32r, tag=f"w0_{k}", name=f"w0_{k}")
                nc.sync.dma_start(out=wt, in_=Wq[k * P:(k + 1) * P, :])
                wsb[(0, k)] = wt
                t = xin_pool.tile([P, N], f32r, tag=f"xqT{k}", name=f"xqT{k}")
                nc.sync.dma_start(out=t, in_=xqT_d[0, k * P:(k + 1) * P, :])
                xqT0.append(t)
            state[(0, "xqT0")] = xqT0

            bo_row = consts.tile([1, D], f32, tag="bo_row")
            nc.sync.dma_start(out=bo_row, in_=bo[:])
            bo_bc = consts.tile([P, D], f32, tag="bo_bc")
            nc.gpsimd.partition_broadcast(bo_bc, bo_row[0:1, :], channels=P)

            # ---- pipelined prep slices ----
            def prep_slice(b, sl):
                if sl == 0:
                    mb = small.tile([P, l_chunks], f32, tag="mbias")
                    nc.sync.dma_start(
                        out=mb, in_=mb_d[b, :].rearrange("(i p) -> p i", p=P))
                    state[(b, "mbias")] = mb
                    xqT = state.pop((0, "xqT0"), None) if b == 0 else None
                    if xqT is None:
                        xqT = []
                        for k in range(NDC):
                            t = xin_pool.tile([P, N], f32r, tag=f"xqT{k}",
                                              name=f"xqT{k}")
                            nc.sync.dma_start(out=t,
                                              in_=xqT_d[b, k * P:(k + 1) * P, :])
                            xqT.append(t)
                    qT = []
                    for j in range(NDC):
                        pq = ps_mm.tile([P, N], f32, tag="mm", name="pq")
                        for k in range(NDC):
                            nc.tensor.matmul(pq, wsb[(0, k)][:, j * P:(j + 1) * P],
                                             xqT[k], start=(k == 0),
                                             stop=(k == NDC - 1))
                        t = qkt_pool.tile([P, N], f32r, tag=f"qT{j}", name=f"qT{j}")
                        nc.vector.tensor_copy(t, pq)
                        qT.append(t)
                    state[(b, "qT")] = qT
                elif sl == 1:
                    xkT, rpbT = [], []
                    for k in range(NDC):
                        t = xin_pool.tile([P, L_C], f32r, tag=f"xkT{k}",
                                          name=f"xkT{k}")
                        nc.sync.dma_start(out=t, in_=xkT_d[b, k * P:(k + 1) * P, :])
                        xkT.append(t)
                    for k in range(NDC):
                        r = xin_pool.tile([P, L_C], mybir.dt.bfloat16, tag=f"rpbT{k}",
                                          name=f"rpbT{k}")
                        nc.sync.dma_start(out=r, in_=rpbT_d[b, k * P:(k + 1) * P, :])
                        rpbT.append(r)
                    state[(b, "xkT")] = xkT
                    state[(b, "rpbT")] = rpbT
                elif sl == 2:
                    xkT = state[(b, "xkT")]
                    rpbT = state.pop((b, "rpbT"))
                    kT = []
                    for j in range(NDC):
                        t = qkt_pool.tile([P, L_C], f32r, tag=f"kT{j}", name=f"kT{j}")
                        for (n0, n1) in _nspans(L_C):
                            pk = ps_mm.tile([P, N], f32, tag="mm", name="pk")
                            for k in range(NDC):
                                nc.tensor.matmul(
                                    pk[:, 0:n1 - n0],
                                    wsb[(1, k)][:, j * P:(j + 1) * P],
                                    xkT[k][:, n0:n1],
                                    start=(k == 0), stop=(k == NDC - 1))
                            nc.vector.tensor_add(
                                t[:, n0:n1], pk[:, 0:n1 - n0], rpbT[j][:, n0:n1])
                        kT.append(t)
                    state[(b, "kT")] = kT
                elif sl == 3:
                    xkT = state.pop((b, "xkT"))
                    vP = []
                    for i in range(l_chunks):
                        pv = ps_mm.tile([P, N], f32, tag="mm", name="pv")
                        for k in range(NDC):
                            nc.tensor.matmul(pv, xkT[k][:, i * P:(i + 1) * P],
                                             wsb[(2, k)], start=(k == 0),
                                             stop=(k == NDC - 1))
                        t = vp_pool.tile([P, H, C + 1], f32r, tag=f"vp{i}",
                                         name=f"vp{i}")
                        nc.vector.tensor_copy(
                            t[:, :, 0:C], pv.rearrange("p (h c) -> p h c", h=H))
                        nc.vector.tensor_copy(t[:, :, C:C + 1], ones8[:, :, None])
                        vP.append(t)
                    state[(b, "vP")] = vP

            def attention_pair(b, j):
                mb = state[(b, "mbias")]
                qT, kT, vP = state[(b, "qT")], state[(b, "kT")], state[(b, "vP")]
                oT = state[(b, "oT")]
                ptiles = []
                for i in range(l_chunks):
                    pss = ps_sc.tile([P, 2 * N], f32, tag="sc", name="pss")
                    for half in range(2):
                        lo = 64 * half
                        nc.tensor.matmul(
                            pss[:, half * N:(half + 1) * N],
                            kT[j][lo:lo + 64, i * P:(i + 1) * P],
                            qT[j][lo:lo + 64, :], start=True, stop=True,
                            tile_position=(lo, 0))
                    pe = pt_pool.tile([P, 2 * N], f32r, tag="pt", name="pe")
                    nc.scalar.activation(out=pe, in_=pss, func=EXP,
                                         bias=mb[:, i:i + 1], scale=SCALE)
                    ptiles.append(pe)
                # stage2, heads interleaved so each ptile releases after 2 reads
                po = {}
                for half in range(2):
                    po[half] = ps_mm.tile([C + 1, N], f32, tag="mm", name="po")
                for i in range(l_chunks):
                    for half in range(2):
                        nc.tensor.matmul(po[half], vP[i][:, 2 * j + half, :],
                                         ptiles[i][:, half * N:(half + 1) * N],
                                         start=(i == 0), stop=(i == l_chunks - 1))
                for half in range(2):
                    tr = small.tile([1, N], f32, tag="tr")
                    nc.vector.reciprocal(tr, po[half][C:C + 1, :])
                    trb = small.tile([C, N], f32, tag="trb")
                    nc.gpsimd.partition_broadcast(trb, tr[0:1, :], channels=C)
                    lo = 64 * half
                    nc.vector.tensor_mul(oT[j][lo:lo + 64, :], po[half][0:C, :], trb)

            def oproj(b):
                oT = state[(b, "oT")]
                for m in range(NNC):
                    pf = ps_mm.tile([P, N], f32, tag="mm", name="pf")
                    for j in range(NDC):
                        nc.tensor.matmul(pf, oT[j][:, m * P:(m + 1) * P],
                                         wsb[(3, j)], start=(j == 0),
                                         stop=(j == NDC - 1))
                    to = outst_pool.tile([P, D], f32, tag="outst", name="to")
                    nc.vector.tensor_add(to, pf, bo_bc)
                    nc.sync.dma_start(out=out[b, m * P:(m + 1) * P, :], in_=to)

            # ---- main pipeline ----
            prep_slice(0, 0)
            load_w(1, Wk)
            prep_slice(0, 1)
            load_w(2, Wv)
            prep_slice(0, 2)
            prep_slice(0, 3)
            load_w(3, Wo)
            for b in range(BLOC):
                state[(b, "oT")] = [
                    ot_pool.tile([P, N], f32r, tag=f"oT{j}", name=f"oT{j}")
                    for j in range(NDC)]
                for j in range(NDC):
                    attention_pair(b, j)
                    if b + 1 < BLOC:
                        prep_slice(b + 1, j)
                oproj(b)

    nc.compile()
    return nc


def _get_nc(l_chunks=LC_SPARSE // P):
    key = ("nc", l_chunks)
    if key not in _CACHE:
        _CACHE[key] = _build_nc(l_chunks)
    return _CACHE[key]


def kernel(x_q, x_kv, pad_mask, Wq, Wk, Wv, Wo, bo, rpb):
    from concourse.bass_utils import run_bass_kernel_spmd

    import ml_dtypes

    x_q = np.asarray(x_q, dtype=np.float32)
    x_kv = np.asarray(x_kv, dtype=np.float32)
    pad_mask = np.asarray(pad_mask).astype(bool)
    rpb2 = np.asarray(rpb, np.float32).reshape(L, D)

    counts = (~pad_mask).sum(axis=1)
    L_C = LC_SPARSE if counts.max() <= LC_SPARSE else L
    nc = _get_nc(L_C // P)

    shared = {
        "Wq": np.asarray(Wq, np.float32), "Wk": np.asarray(Wk, np.float32),
        "Wv": np.asarray(Wv, np.float32), "Wo": np.asarray(Wo, np.float32),
        "bo": np.asarray(bo, np.float32).reshape(1, D),
    }
    in_maps = []
    for c in range(NCORES):
        sl = slice(c * BLOC, (c + 1) * BLOC)
        xkT = np.zeros((BLOC, D, L_C), np.float32)
        rpbT = np.zeros((BLOC, D, L_C), np.float32)
        mb = np.full((BLOC, L_C), MASK_NEG, np.float32)
        for b in range(BLOC):
            g = c * BLOC + b
            idx = np.nonzero(~pad_mask[g])[0]
            cnt = len(idx)
            xkT[b, :, :cnt] = x_kv[g, idx, :].T
            rpbT[b, :, :cnt] = rpb2[idx, :].T
            mb[b, :cnt] = 0.0
        in_maps.append({
            "xqT": np.ascontiguousarray(x_q[sl].transpose(0, 2, 1)),
            "xkT": xkT, "rpbT": rpbT.astype(ml_dtypes.bfloat16), "mbias": mb,
            **shared,
        })
    res = run_bass_kernel_spmd(nc, in_maps, list(range(NCORES)))
    return np.concatenate([res.results[c]["out"] for c in range(NCORES)], axis=0)



# revision 12
# speedup vs baseline: 1.0625x; 1.0625x over previous
"""MultiHeadAttention (cross-attention, B=32 N=512 L=1024 D=512 H=8) on 8 TRN2 cores.

Strategy: data parallelism (4 batches/core) + host-side sparsity compaction.

Host prep (inside kernel(), plain numpy):
  - per batch, gather the unmasked K/V positions (~50% of L=1024), pad to
    L_C=640 (5*128); padded slots get zero K/V rows and a -87 exp bias so they
    vanish from the softmax exactly like reference's -inf masking
  - rpb rows gathered the same way; x_q / x_kv / rpb pre-TRANSPOSED on host so
    the device needs no PE transposes at all
Device per-core dataflow (all matmuls float32r, 1 cycle/row on PE):
  Q^T/K^T (+rpb^T via DVE add) head-major; V natural with interleaved ones col
  scores S^T[l,n] per head-pair packed via tile_position (K=64 row groups),
  both heads' scores in one [128,1024] PSUM tile -> single exp per (pair,chunk)
  exp on ACT with per-partition bias (pad masking; no max subtraction needed)
  stage2 O^T[c,n] = [V|1]^T @ P^T accumulated over l chunks (heads interleaved
  so P^T tiles release early); row 64 = softmax denominator
  normalize via reciprocal + gpsimd partition_broadcast, o_proj to natural
  layout, + bias, DMA out.
Emission is software-pipelined: prep (DMAs + QKV projections) of batch b+1 is
interleaved into the ACT-bound attention phase of batch b.
"""
import sys

sys.path.insert(0, "/opt/trn_rl_repo")
import numpy as np

B, N, L, D, H, C = 32, 512, 1024, 512, 8, 64
NCORES = 8
BLOC = B // NCORES  # 4 batches per core
SCALE = C ** -0.5
MASK_NEG = -87.0
P = 128
NDC = D // P   # 4 d/e chunks
NNC = N // P   # 4 n chunks
LC_SPARSE = 640

_CACHE = {}


def _nspans(l_c):
    # split l_c into moving-operand spans <=512, each >=256 (f32r full rate)
    if l_c == 640:
        return [(0, 384), (384, 640)]
    return [(s, min(s + 512, l_c)) for s in range(0, l_c, 512)]


def _build_nc(l_chunks):
    import concourse.bacc as bacc
    import concourse.tile as tile
    from concourse import mybir

    f32 = mybir.dt.float32
    f32r = mybir.dt.float32r
    fp8 = mybir.dt.float8e4
    DR = mybir.MatmulPerfMode.DoubleRow
    EXP = mybir.ActivationFunctionType.Exp
    L_C = l_chunks * P
    n_pair = l_chunks // 2
    has_rem = l_chunks % 2

    nc = bacc.Bacc()
    xqT_d = nc.declare_dram_parameter("xqT", [BLOC, D, N], f32r, isOutput=False)
    xkT_d = nc.declare_dram_parameter("xkT", [BLOC, D, L_C], f32r, isOutput=False)
    rpbT_d = nc.declare_dram_parameter("rpbT", [BLOC, D, L_C], mybir.dt.bfloat16, isOutput=False)
    mb_d = nc.declare_dram_parameter("mbias", [BLOC, L_C], f32, isOutput=False)
    Wq = nc.declare_dram_parameter("Wq", [D, D], f32r, isOutput=False)
    Wk = nc.declare_dram_parameter("Wk", [D, D], f32r, isOutput=False)
    Wv = nc.declare_dram_parameter("Wv", [D, D], f32r, isOutput=False)
    Wo = nc.declare_dram_parameter("Wo", [D, D], f32r, isOutput=False)
    bo = nc.declare_dram_parameter("bo", [1, D], f32, isOutput=False)
    out = nc.declare_dram_parameter("out", [BLOC, N, D], f32, isOutput=True)

    sparse = l_chunks <= 5
    with tile.TileContext(nc) as tc:
        with (
            tc.tile_pool(name="consts", bufs=1) as consts,
            tc.tile_pool(name="xin", bufs=2 if sparse else 1) as xin_pool,
            tc.tile_pool(name="qkt", bufs=2 if sparse else 1) as qkt_pool,
            tc.tile_pool(name="vp", bufs=2 if sparse else 1) as vp_pool,
            tc.tile_pool(name="pt", bufs=2) as pt_pool,
            tc.tile_pool(name="ot", bufs=2) as ot_pool,
            tc.tile_pool(name="outst", bufs=3) as outst_pool,
            tc.tile_pool(name="small", bufs=2) as small,
            tc.tile_pool(name="ps_sc", bufs=2, space="PSUM") as ps_sc,
            tc.tile_pool(name="ps_mm", bufs=4, space="PSUM") as ps_mm,
        ):
            state = {}

            # ---- one-time setup ----
            warm = consts.tile([P, 1], f32, tag="warm")
            nc.vector.memset(warm, 0.0)
            nc.scalar.activation(out=warm, in_=warm, func=EXP, scale=1.0)

            ones8 = consts.tile([P, H], f32, tag="ones8")
            nc.vector.memset(ones8, 1.0)

            wsb = {}

            def load_w(wi, W):
                for k in range(NDC):
                    wt = consts.tile([P, D], f32r, tag=f"w{wi}_{k}", name=f"w{wi}_{k}")
                    nc.sync.dma_start(out=wt, in_=W[k * P:(k + 1) * P, :])
                    wsb[(wi, k)] = wt

            # Wq and batch-0 xqT interleaved by chunk so the first Q-proj
            # matmul group can start after ~0.5 MB of DMA
            xqT0 = []
            for k in range(NDC):
                wt = consts.tile([P, D], f32r, tag=f"w0_{k}", name=f"w0_{k}")
                nc.sync.dma_start(out=wt, in_=Wq[k * P:(k + 1) * P, :])
                wsb[(0, k)] = wt
                t = xin_pool.tile([P, N], f32r, tag=f"xqT{k}", name=f"xqT{k}")
                nc.sync.dma_start(out=t, in_=xqT_d[0, k * P:(k + 1) * P, :])
                xqT0.append(t)
            state[(0, "xqT0")] = xqT0

            bo_row = consts.tile([1, D], f32, tag="bo_row")
            nc.sync.dma_start(out=bo_row, in_=bo[:])
            bo_bc = consts.tile([P, D], f32, tag="bo_bc")
            nc.gpsimd.partition_broadcast(bo_bc, bo_row[0:1, :], channels=P)

            # ---- pipelined prep slices ----
            def prep_slice(b, sl):
                if sl == 0:
                    mb = small.tile([P, l_chunks], f32, tag="mbias")
                    nc.sync.dma_start(
                        out=mb, in_=mb_d[b, :].rearrange("(i p) -> p i", p=P))
                    state[(b, "mbias")] = mb
                    xqT = state.pop((0, "xqT0"), None) if b == 0 else None
                    if xqT is None:
                        xqT = []
                        for k in range(NDC):
                            t = xin_pool.tile([P, N], f32r, tag=f"xqT{k}",
                                              name=f"xqT{k}")
                            nc.sync.dma_start(out=t,
                                              in_=xqT_d[b, k * P:(k + 1) * P, :])
                            xqT.append(t)
                    qT = []
                    for j in range(NDC):
                        pq = ps_mm.tile([P, N], f32, tag="mm", name="pq")
                        for k in range(NDC):
                            nc.tensor.matmul(pq, wsb[(0, k)][:, j * P:(j + 1) * P],
                                             xqT[k], start=(k == 0),
                                             stop=(k == NDC - 1))
                        t = qkt_pool.tile([P, N], f32r, tag=f"qT{j}", name=f"qT{j}")
                        nc.vector.tensor_copy(t, pq)
                        qT.append(t)
                    state[(b, "qT")] = qT
                elif sl == 1:
                    xkT, rpbT = [], []
                    for k in range(NDC):
                        t = xin_pool.tile([P, L_C], f32r, tag=f"xkT{k}",
                                          name=f"xkT{k}")
                        nc.sync.dma_start(out=t, in_=xkT_d[b, k * P:(k + 1) * P, :])
                        xkT.append(t)
                    for k in range(NDC):
                        r = xin_pool.tile([P, L_C], mybir.dt.bfloat16, tag=f"rpbT{k}",
                                          name=f"rpbT{k}")
                        nc.sync.dma_start(out=r, in_=rpbT_d[b, k * P:(k + 1) * P, :])
                        rpbT.append(r)
                    state[(b, "xkT")] = xkT
                    state[(b, "rpbT")] = rpbT
                elif sl == 2:
                    xkT = state[(b, "xkT")]
                    rpbT = state.pop((b, "rpbT"))
                    kT = []
                    for j in range(NDC):
                        t = qkt_pool.tile([P, L_C], f32r, tag=f"kT{j}", name=f"kT{j}")
                        for (n0, n1) in _nspans(L_C):
                            pk = ps_mm.tile([P, N], f32, tag="mm", name="pk")
                            for k in range(NDC):
                                nc.tensor.matmul(
                                    pk[:, 0:n1 - n0],
                                    wsb[(1, k)][:, j * P:(j + 1) * P],
                                    xkT[k][:, n0:n1],
                                    start=(k == 0), stop=(k == NDC - 1))
                            nc.vector.tensor_add(
                                t[:, n0:n1], pk[:, 0:n1 - n0], rpbT[j][:, n0:n1])
                        kT.append(t)
                    state[(b, "kT")] = kT
                elif sl == 3:
                    xkT = state.pop((b, "xkT"))
                    vP = []
                    for i in range(l_chunks):
                        # dim1: 0 = fp8(V) | ones col, 1 = fp8 residual | zeros.
                        # Both halves feed one DoubleRow matmul (2 k-tiles), so
                        # V is effectively kept at ~fp16 precision while the
                        # moving P tile streams at fp8 DoubleRow rate.
                        t = vp_pool.tile([P, 2, H, C + 2], fp8, tag=f"vp{i}",
                                         name=f"vp{i}")
                        pv = ps_mm.tile([P, N], f32, tag="mm", name="pv")
                        for k in range(NDC):
                            nc.tensor.matmul(pv, xkT[k][:, i * P:(i + 1) * P],
                                             wsb[(2, k)], start=(k == 0),
                                             stop=(k == NDC - 1))
                        pvh = pv.rearrange("p (h c) -> p h c", h=H)
                        nc.vector.tensor_copy(t[:, 0, :, 0:C], pvh)
                        nc.vector.scalar_tensor_tensor(
                            out=t[:, 1, :, 0:C], in0=pvh, scalar=1.0,
                            in1=t[:, 0, :, 0:C], op0=mybir.AluOpType.mult,
                            op1=mybir.AluOpType.subtract)
                        nc.vector.tensor_copy(t[:, 0, :, C:C + 1],
                                              ones8[:, :, None])
                        nc.vector.memset(t[:, 1, :, C:C + 1], 0.0)
                        vP.append(t)
                    state[(b, "vP")] = vP

            def attention_pair(b, j):
                mb = state[(b, "mbias")]
                qT, kT, vP = state[(b, "qT")], state[(b, "kT")], state[(b, "vP")]
                oT = state[(b, "oT")]
                ptiles = []
                for i in range(l_chunks):
                    pt = pt_pool.tile([P, 2 * N], fp8, tag=f"pt{i}",
                                      name=f"pt{i}")
                    pss = ps_sc.tile([P, 2 * N], f32, tag="sc", name="pss")
                    for half in range(2):
                        lo = 64 * half
                        nc.tensor.matmul(
                            pss[:, half * N:(half + 1) * N],
                            kT[j][lo:lo + 64, i * P:(i + 1) * P],
                            qT[j][lo:lo + 64, :], start=True, stop=True,
                            tile_position=(lo, 0))
                    nc.scalar.activation(out=pt, in_=pss, func=EXP,
                                         bias=mb[:, i:i + 1], scale=SCALE)
                    ptiles.append(pt)
                # stage2: fp8 DoubleRow; k-tile pair = (V_hi, V_lo) against the
                # SAME P tile (stride-0 broadcast on the moving operand). Heads
                # interleaved so each pt tile releases after 2 reads.
                po = {}
                for half in range(2):
                    po[half] = ps_mm.tile([C + 1, N], f32, tag="mm", name="po")
                for i in range(l_chunks):
                    for half in range(2):
                        pmov = (ptiles[i][:, half * N:(half + 1) * N]
                                .unsqueeze(1).broadcast_to([P, 2, N]))
                        nc.tensor.matmul(
                            po[half], vP[i][:, :, 2 * j + half, 0:C + 1],
                            pmov, start=(i == 0), stop=(i == l_chunks - 1),
                            perf_mode=DR)
                for half in range(2):
                    tr = small.tile([1, N], f32, tag="tr")
                    nc.vector.reciprocal(tr, po[half][C:C + 1, :])
                    trb = small.tile([C, N], f32, tag="trb")
                    nc.gpsimd.partition_broadcast(trb, tr[0:1, :], channels=C)
                    lo = 64 * half
                    nc.vector.tensor_mul(oT[j][lo:lo + 64, :], po[half][0:C, :], trb)

            def oproj(b):
                oT = state[(b, "oT")]
                for m in range(NNC):
                    pf = ps_mm.tile([P, N], f32, tag="mm", name="pf")
                    for j in range(NDC):
                        nc.tensor.matmul(pf, oT[j][:, m * P:(m + 1) * P],
                                         wsb[(3, j)], start=(j == 0),
                                         stop=(j == NDC - 1))
                    to = outst_pool.tile([P, D], f32, tag="outst", name="to")
                    nc.vector.tensor_add(to, pf, bo_bc)
                    nc.sync.dma_start(out=out[b, m * P:(m + 1) * P, :], in_=to)

            # ---- main pipeline ----
            prep_slice(0, 0)
            load_w(1, Wk)
            prep_slice(0, 1)
            load_w(2, Wv)
            prep_slice(0, 2)
            prep_slice(0, 3)
            load_w(3, Wo)
            for b in range(BLOC):
                state[(b, "oT")] = [
                    ot_pool.tile([P, N], f32r, tag=f"oT{j}", name=f"oT{j}")
                    for j in range(NDC)]
                for j in range(NDC):
                    attention_pair(b, j)
                    if b + 1 < BLOC:
                        prep_slice(b + 1, j)
                oproj(b)

    nc.compile()
    return nc


def _get_nc(l_chunks=LC_SPARSE // P):
    key = ("nc", l_chunks)
    if key not in _CACHE:
        _CACHE[key] = _build_nc(l_chunks)
    return _CACHE[key]


def kernel(x_q, x_kv, pad_mask, Wq, Wk, Wv, Wo, bo, rpb):
    from concourse.bass_utils import run_bass_kernel_spmd

    import ml_dtypes

    x_q = np.asarray(x_q, dtype=np.float32)
    x_kv = np.asarray(x_kv, dtype=np.float32)
    pad_mask = np.asarray(pad_mask).astype(bool)
    rpb2 = np.asarray(rpb, np.float32).reshape(L, D)

    counts = (~pad_mask).sum(axis=1)
    L_C = LC_SPARSE if counts.max() <= LC_SPARSE else L
    nc = _get_nc(L_C // P)

    shared = {
        "Wq": np.asarray(Wq, np.float32), "Wk": np.asarray(Wk, np.float32),
        "Wv": np.asarray(Wv, np.float32), "Wo": np.asarray(Wo, np.float32),
        "bo": np.asarray(bo, np.float32).reshape(1, D),
    }
    in_maps = []
    for c in range(NCORES):
        sl = slice(c * BLOC, (c + 1) * BLOC)
        xkT = np.zeros((BLOC, D, L_C), np.float32)
        rpbT = np.zeros((BLOC, D, L_C), np.float32)
        mb = np.full((BLOC, L_C), MASK_NEG, np.float32)
        for b in range(BLOC):
            g = c * BLOC + b
            idx = np.nonzero(~pad_mask[g])[0]
            cnt = len(idx)
            xkT[b, :, :cnt] = x_kv[g, idx, :].T
            rpbT[b, :, :cnt] = rpb2[idx, :].T
            # -4 shift keeps unnormalized exp inside fp8e4m3 range (max
            # observed scaled score is ~7.7; e^(7.7-4) ~= 40 << 240); the
            # shift cancels in softmax
            mb[b, :cnt] = -4.0
        in_maps.append({
            "xqT": np.ascontiguousarray(x_q[sl].transpose(0, 2, 1)),
            "xkT": xkT, "rpbT": rpbT.astype(ml_dtypes.bfloat16), "mbias": mb,
            **shared,
        })
    res = run_bass_kernel_spmd(nc, in_maps, list(range(NCORES)))
    return np.concatenate([res.results[c]["out"] for c in range(NCORES)], axis=0)



# revision 15
# speedup vs baseline: 1.1179x; 1.0521x over previous
"""MultiHeadAttention (cross-attention, B=32 N=512 L=1024 D=512 H=8) on 8 TRN2 cores.

Strategy: data parallelism (4 batches/core) + host-side sparsity compaction.

Host prep (inside kernel(), plain numpy):
  - per batch, gather the unmasked K/V positions (~50% of L=1024), pad to
    L_C=640 (5*128); padded slots get zero K/V rows and a -87 exp bias so they
    vanish from the softmax exactly like reference's -inf masking
  - x_q / x_kv pre-transposed AND pre-split into fp8e4m3 hi+lo pairs (hi =
    fp8(x), lo = fp8(x - hi)); Wq/Wk/Wv likewise; Wo in bf16
Device per-core dataflow:
  Q/K/V projections as fp8 DoubleRow matmuls, 3 term chains
  (x_hi*W_hi + x_hi*W_lo + x_lo*W_hi; dropped lo*lo term is ~0.07%) -> 0.75x
  precision-preserving at 4x DoubleRow rate = 3x faster than f32r.
  scores S^T[l,n] per head-pair packed via tile_position (K=64 row groups),
  both heads' scores in one [128,1024] f32 PSUM tile (f32r matmuls, exact)
  exp on ACT with per-partition bias (pad masking, -4 shift for fp8 range),
  fp8e4m3 output, l-chunk-paired tiles
  stage2 emits O[n,c] PER HEAD (M=n): stationary = P^T l-chunk pairs
  (DoubleRow k-tiles), moving = V|ones fp8; TWO chains (V_hi, V_lo residual)
  keep V at ~fp16 precision; out free dim is only 66 -> 33 cycles/matmul.
  denominator rides along as a ones column -> per-PARTITION reciprocal +
  broadcast multiply (cheap), PE-transpose (bf16) back to O^T[d,n] for o_proj
  o_proj in bf16, + bias, DMA out.
Emission is software-pipelined: prep (DMAs + QKV projections) of batch b+1 is
interleaved into the attention phase of batch b.
"""
import sys

sys.path.insert(0, "/opt/trn_rl_repo")
import numpy as np

B, N, L, D, H, C = 32, 512, 1024, 512, 8, 64
NCORES = 8
BLOC = B // NCORES  # 4 batches per core
SCALE = C ** -0.5
MASK_NEG = -87.0
EXP_SHIFT = -4.0  # keeps unnormalized exp inside fp8e4m3 range (max scaled
                  # score ~7.7 -> e^3.7 ~= 40 << 240); cancels in softmax
W_PRESCALE = 32.0  # lifts W out of fp8 subnormal range; q,k,v scaled by 32
P = 128
NDC = D // P   # 4 d/e chunks
NNC = N // P   # 4 n chunks
LC_SPARSE = 640

_CACHE = {}


def _nspans(l_c):
    # PSUM bank is 512 f32 -> split K-proj output into spans <=512
    if l_c == 640:
        return [(0, 384), (384, 640)]
    return [(s, min(s + 512, l_c)) for s in range(0, l_c, 512)]


def _build_nc(l_chunks):
    import concourse.bacc as bacc
    import concourse.tile as tile
    from concourse import mybir

    f32 = mybir.dt.float32
    f32r = mybir.dt.float32r
    bf16 = mybir.dt.bfloat16
    fp8 = mybir.dt.float8e4
    DR = mybir.MatmulPerfMode.DoubleRow
    EXP = mybir.ActivationFunctionType.Exp
    MUL = mybir.AluOpType.mult
    SUB = mybir.AluOpType.subtract
    L_C = l_chunks * P
    SC_EXP = SCALE / (W_PRESCALE * W_PRESCALE)
    # fp8 term chains: (x_hi*W_hi), (x_hi*W_lo), (x_lo*W_hi)
    TERMS = ((0, 0), (0, 1), (1, 0))

    nc = bacc.Bacc()
    xq8_d = nc.declare_dram_parameter("xq8", [BLOC, 2, P, NDC, N], fp8, isOutput=False)
    xk8_d = nc.declare_dram_parameter("xk8", [BLOC, 2, P, NDC, L_C], fp8, isOutput=False)
    rpbT_d = nc.declare_dram_parameter("rpbT", [BLOC, D, L_C], bf16, isOutput=False)
    mb_d = nc.declare_dram_parameter("mbias", [BLOC, L_C], f32, isOutput=False)
    Wq8_d = nc.declare_dram_parameter("Wq8", [2, P, NDC, D], fp8, isOutput=False)
    Wk8_d = nc.declare_dram_parameter("Wk8", [2, P, NDC, D], fp8, isOutput=False)
    Wv8_d = nc.declare_dram_parameter("Wv8", [2, P, NDC, D], fp8, isOutput=False)
    Wo16_d = nc.declare_dram_parameter("Wo16", [D, D], bf16, isOutput=False)
    id_d = nc.declare_dram_parameter("ident", [P, P], bf16, isOutput=False)
    bo = nc.declare_dram_parameter("bo", [1, D], f32, isOutput=False)
    out = nc.declare_dram_parameter("out", [BLOC, N, D], f32, isOutput=True)

    with tile.TileContext(nc) as tc:
        with (
            tc.tile_pool(name="consts", bufs=1) as consts,
            tc.tile_pool(name="xin", bufs=2) as xin_pool,
            tc.tile_pool(name="qkt", bufs=2) as qkt_pool,
            tc.tile_pool(name="vp", bufs=2) as vp_pool,
            tc.tile_pool(name="pt", bufs=2) as pt_pool,
            tc.tile_pool(name="ot", bufs=2) as ot_pool,
            tc.tile_pool(name="outst", bufs=3) as outst_pool,
            tc.tile_pool(name="small", bufs=2) as small,
            tc.tile_pool(name="ps_sc", bufs=2, space="PSUM") as ps_sc,
            tc.tile_pool(name="ps_mm", bufs=2, space="PSUM") as ps_mm,
            tc.tile_pool(name="ps_po", bufs=2, space="PSUM") as ps_po,
        ):
            state = {}

            # ---- one-time setup ----
            warm = consts.tile([P, 1], f32, tag="warm")
            nc.vector.memset(warm, 0.0)
            nc.scalar.activation(out=warm, in_=warm, func=EXP, scale=1.0)

            ones8 = consts.tile([P, H], f32, tag="ones8")
            nc.vector.memset(ones8, 1.0)

            wsb = {}

            # Wq(hi) and batch-0 xq(hi) first so Q-proj term 1 can start early
            wq = {}
            for hl in range(2):
                wt = consts.tile([P, NDC, D], fp8, tag=f"wq{hl}", name=f"wq{hl}")
                nc.sync.dma_start(out=wt, in_=Wq8_d[hl])
                wq[hl] = wt
                t = xin_pool.tile([P, NDC, N], fp8, tag=f"xq{hl}", name=f"xq{hl}")
                nc.sync.dma_start(out=t, in_=xq8_d[0, hl])
                state[(0, f"xq{hl}")] = t
            wsb["q"] = wq

            def load_w8(key, Wd):
                ws = {}
                for hl in range(2):
                    wt = consts.tile([P, NDC, D], fp8, tag=f"w{key}{hl}",
                                     name=f"w{key}{hl}")
                    nc.sync.dma_start(out=wt, in_=Wd[hl])
                    ws[hl] = wt
                wsb[key] = ws

            def load_wo():
                for k in range(NDC):
                    wt = consts.tile([P, D], bf16, tag=f"wo{k}", name=f"wo{k}")
                    nc.sync.dma_start(out=wt, in_=Wo16_d[k * P:(k + 1) * P, :])
                    wsb[("o", k)] = wt
                idt = consts.tile([P, P], bf16, tag="idt")
                nc.sync.dma_start(out=idt, in_=id_d[:])
                wsb["idt"] = idt
                bo_row = consts.tile([1, D], f32, tag="bo_row")
                nc.sync.dma_start(out=bo_row, in_=bo[:])
                bo_bc = consts.tile([P, D], f32, tag="bo_bc")
                nc.gpsimd.partition_broadcast(bo_bc, bo_row[0:1, :], channels=P)
                wsb["bo"] = bo_bc

            def proj_dr(pacc, ws, xs, stat_sel, mov_sel, x_stat):
                """Emit the 6-matmul fp8 DoubleRow projection group."""
                nt = len(TERMS)
                for ti, (xi, wi) in enumerate(TERMS):
                    for u in range(NDC // 2):
                        ku = slice(2 * u, 2 * u + 2)
                        if x_stat:
                            lhsT = xs[xi][:, ku, stat_sel]
                            rhs = ws[wi][:, ku, mov_sel]
                        else:
                            lhsT = ws[wi][:, ku, stat_sel]
                            rhs = xs[xi][:, ku, mov_sel]
                        nc.tensor.matmul(
                            pacc, lhsT, rhs,
                            start=(ti == 0 and u == 0),
                            stop=(ti == nt - 1 and u == NDC // 2 - 1),
                            perf_mode=DR)

            # ---- pipelined prep slices ----
            def prep_slice(b, sl):
                if sl == 0:
                    mb = small.tile([P, l_chunks], f32, tag="mbias")
                    nc.sync.dma_start(
                        out=mb, in_=mb_d[b, :].rearrange("(i p) -> p i", p=P))
                    state[(b, "mbias")] = mb
                    xq = {}
                    for hl in range(2):
                        t = state.pop((0, f"xq{hl}"), None) if b == 0 else None
                        if t is None:
                            t = xin_pool.tile([P, NDC, N], fp8, tag=f"xq{hl}",
                                              name=f"xq{hl}")
                            nc.sync.dma_start(out=t, in_=xq8_d[b, hl])
                        xq[hl] = t
                    qT = []
                    for j in range(NDC):
                        pq = ps_mm.tile([P, N], f32, tag="mm", name="pq")
                        proj_dr(pq, wsb["q"], xq,
                                slice(j * P, (j + 1) * P), slice(0, N), False)
                        t = qkt_pool.tile([P, N], f32r, tag=f"qT{j}", name=f"qT{j}")
                        nc.vector.tensor_copy(t, pq)
                        qT.append(t)
                    state[(b, "qT")] = qT
                elif sl == 1:
                    xk = {}
                    for hl in range(2):
                        t = xin_pool.tile([P, NDC, L_C], fp8, tag=f"xk{hl}",
                                          name=f"xk{hl}")
                        nc.sync.dma_start(out=t, in_=xk8_d[b, hl])
                        xk[hl] = t
                    state[(b, "xk")] = xk
                    rpbT = []
                    for k in range(NDC):
                        r = xin_pool.tile([P, L_C], bf16, tag=f"rpbT{k}",
                                          name=f"rpbT{k}")
                        nc.sync.dma_start(out=r, in_=rpbT_d[b, k * P:(k + 1) * P, :])
                        rpbT.append(r)
                    state[(b, "rpbT")] = rpbT
                elif sl == 2:
                    xk = state[(b, "xk")]
                    rpbT = state.pop((b, "rpbT"))
                    kT = []
                    for j in range(NDC):
                        t = qkt_pool.tile([P, L_C], f32r, tag=f"kT{j}", name=f"kT{j}")
                        for (n0, n1) in _nspans(L_C):
                            pk = ps_mm.tile([P, N], f32, tag="mm", name="pk")
                            proj_dr(pk[:, 0:n1 - n0], wsb["k"], xk,
                                    slice(j * P, (j + 1) * P), slice(n0, n1), False)
                            nc.vector.tensor_add(
                                t[:, n0:n1], pk[:, 0:n1 - n0], rpbT[j][:, n0:n1])
                        kT.append(t)
                    state[(b, "kT")] = kT
                elif sl == 3:
                    xk = state.pop((b, "xk"))
                    vP = []
                    for i in range(l_chunks):
                        t = vp_pool.tile([P, H, C + 2], bf16, tag=f"vp{i}",
                                         name=f"vp{i}")
                        pv = ps_mm.tile([P, N], f32, tag="mm", name="pv")
                        proj_dr(pv, wsb["v"], xk,
                                slice(i * P, (i + 1) * P), slice(0, D), True)
                        nc.vector.tensor_copy(
                            t[:, :, 0:C], pv.rearrange("p (h c) -> p h c", h=H))
                        nc.vector.tensor_copy(t[:, :, C:C + 1], ones8[:, :, None])
                        nc.vector.memset(t[:, :, C + 1:C + 2], 0.0)
                        vP.append(t)
                    state[(b, "vP")] = vP

            def attention_pair(b, j):
                mb = state[(b, "mbias")]
                qT, kT, vP = state[(b, "qT")], state[(b, "kT")], state[(b, "vP")]
                oT = state[(b, "oT")]
                ptiles = []
                for i in range(l_chunks):
                    pt = pt_pool.tile([P, 2 * N], bf16, tag=f"pt{i}",
                                      name=f"pt{i}")
                    pss = ps_sc.tile([P, 2 * N], f32, tag="sc", name="pss")
                    for half in range(2):
                        lo = 64 * half
                        nc.tensor.matmul(
                            pss[:, half * N:(half + 1) * N],
                            kT[j][lo:lo + 64, i * P:(i + 1) * P],
                            qT[j][lo:lo + 64, :], start=True, stop=True,
                            tile_position=(lo, 0))
                    nc.scalar.activation(out=pt, in_=pss, func=EXP,
                                         bias=mb[:, i:i + 1], scale=SC_EXP)
                    ptiles.append(pt)
                # stage2: out O[n, c] per head; stationary = P^T l-chunk pairs
                # (DoubleRow k-tiles), moving = V|1 fp8; two chains keep V at
                # ~fp16 precision. Denominator = ones column (col C).
                po = {}
                for he in range(2):
                    po[he] = ps_po.tile([P, NNC, C + 2], f32, tag="po", name="po")
                # PSUM zero-regions are whole 2KB banks: start=True may only
                # be issued ONCE per po tile (it flags the full bank pending-
                # zero); later first-writes to still-pending bytes overwrite.
                for i in range(l_chunks):
                    for he in range(2):
                        for m in range(NNC):
                            sel = slice(he * N + m * P, he * N + (m + 1) * P)
                            nc.tensor.matmul(
                                po[he][:, m, :], ptiles[i][:, sel],
                                vP[i][:, 2 * j + he, :],
                                start=(i == 0 and m == 0),
                                stop=(i == l_chunks - 1 and m == NNC - 1),
                                skip_group_check=True)
                oN = ot_pool.tile([P, NNC, 2, C], bf16, tag="oN", name="oN")
                for he in range(2):
                    rc = small.tile([P, NNC], f32, tag="rc", name="rc")
                    nc.vector.reciprocal(rc, po[he][:, :, C:C + 1].squeeze(2))
                    nc.vector.tensor_mul(
                        oN[:, :, he, :], po[he][:, :, 0:C],
                        rc.unsqueeze(2).broadcast_to([P, NNC, C]))
                for m in range(NNC):
                    ptr = ps_mm.tile([P, P], bf16, tag="mm", name="ptr")
                    nc.tensor.transpose(ptr, oN[:, m, :, :], wsb["idt"])
                    nc.vector.tensor_copy(oT[j][:, m * P:(m + 1) * P], ptr)

            def oproj(b):
                oT = state[(b, "oT")]
                for m in range(NNC):
                    pf = ps_mm.tile([P, N], f32, tag="mm", name="pf")
                    for k in range(NDC):
                        nc.tensor.matmul(pf, oT[k][:, m * P:(m + 1) * P],
                                         wsb[("o", k)], start=(k == 0),
                                         stop=(k == NDC - 1))
                    to = outst_pool.tile([P, D], f32, tag="outst", name="to")
                    nc.vector.tensor_add(to, pf, wsb["bo"])
                    nc.sync.dma_start(out=out[b, m * P:(m + 1) * P, :], in_=to)

            # ---- main pipeline ----
            prep_slice(0, 0)
            load_w8("k", Wk8_d)
            prep_slice(0, 1)
            load_w8("v", Wv8_d)
            prep_slice(0, 2)
            prep_slice(0, 3)
            load_wo()
            for b in range(BLOC):
                state[(b, "oT")] = [
                    ot_pool.tile([P, N], bf16, tag=f"oT{j}", name=f"oT{j}")
                    for j in range(NDC)]
                for j in range(NDC):
                    attention_pair(b, j)
                    if b + 1 < BLOC:
                        prep_slice(b + 1, j)
                oproj(b)

    nc.compile()
    return nc


def _get_nc(l_chunks=LC_SPARSE // P):
    key = ("nc", l_chunks)
    if key not in _CACHE:
        _CACHE[key] = _build_nc(l_chunks)
    return _CACHE[key]


def _fp8_split(x, e4):
    hi = x.astype(e4)
    lo = (x - hi.astype(np.float32)).astype(e4)
    return hi, lo


def kernel(x_q, x_kv, pad_mask, Wq, Wk, Wv, Wo, bo, rpb):
    from concourse.bass_utils import run_bass_kernel_spmd

    import ml_dtypes

    e4 = ml_dtypes.float8_e4m3
    bf16 = ml_dtypes.bfloat16

    x_q = np.asarray(x_q, dtype=np.float32)
    x_kv = np.asarray(x_kv, dtype=np.float32)
    pad_mask = np.asarray(pad_mask).astype(bool)
    rpb2 = np.asarray(rpb, np.float32).reshape(L, D)

    counts = (~pad_mask).sum(axis=1)
    L_C = LC_SPARSE if counts.max() <= LC_SPARSE else L
    lch = L_C // P
    nc = _get_nc(lch)

    def wsplit(W):
        # [2, P, NDC, D]: (hl, p, k, e) = split(32 * W[k*128+p, e])
        Wr = np.asarray(W, np.float32).reshape(NDC, P, D).transpose(1, 0, 2)
        hi, lo = _fp8_split(Wr * W_PRESCALE, e4)
        return np.stack([hi, lo])

    shared = {
        "Wq8": wsplit(Wq), "Wk8": wsplit(Wk), "Wv8": wsplit(Wv),
        "Wo16": (np.asarray(Wo, np.float32) / W_PRESCALE).astype(bf16),
        "ident": np.eye(P, dtype=bf16),
        "bo": np.asarray(bo, np.float32).reshape(1, D),
    }
    in_maps = []
    for c in range(NCORES):
        xq8 = np.zeros((BLOC, 2, P, NDC, N), e4)
        xk8 = np.zeros((BLOC, 2, P, NDC, L_C), e4)
        rpbT = np.zeros((BLOC, D, L_C), np.float32)
        mb = np.full((BLOC, L_C), MASK_NEG, np.float32)
        for b in range(BLOC):
            g = c * BLOC + b
            # xq: (p, k, n) = x_q[g][n, k*128+p]
            xqT = x_q[g].T.reshape(NDC, P, N).transpose(1, 0, 2)
            hi, lo = _fp8_split(xqT, e4)
            xq8[b, 0], xq8[b, 1] = hi, lo
            idx = np.nonzero(~pad_mask[g])[0]
            cnt = len(idx)
            xkT = np.zeros((D, L_C), np.float32)
            xkT[:, :cnt] = x_kv[g, idx, :].T
            xkr = xkT.reshape(NDC, P, L_C).transpose(1, 0, 2)
            hi, lo = _fp8_split(xkr, e4)
            xk8[b, 0], xk8[b, 1] = hi, lo
            rpbT[b, :, :cnt] = W_PRESCALE * rpb2[idx, :].T
            mb[b, :cnt] = EXP_SHIFT
        in_maps.append({
            "xq8": xq8, "xk8": xk8, "rpbT": rpbT.astype(bf16), "mbias": mb,
            **shared,
        })
    res = run_bass_kernel_spmd(nc, in_maps, list(range(NCORES)))
    return np.concatenate([res.results[c]["out"] for c in range(NCORES)], axis=0)
